# revision 29
# baseline (speedup 1.0000x reference)
"""Trainium2 Bass kernel for a 2-layer GAT node classifier (SPMD over 8 NeuronCores).

Strategy (per layer):
  - Replicated dense phase: every core computes the full projection table
    H'[n] = [x @ W | x @ (W B_l)] (h plus the per-head left-attention dot),
    written to per-core HBM gather tables. The right-attention dot er is kept
    only for the core's own destination-node range, resident in SBUF.
  - Edge phase: destination nodes are sharded contiguously across cores
    (6250 per core). Per core, edges sorted by destination, grouped into
    128-node destination blocks, padded to 128-edge tiles, and split into
    "lo"/"hi" source groups so the int16 gather indices can address the
    whole 50048-row table via two base tables.
  - Per 1024-edge gather group (one SWDGE dma_gather): one-hot selection
    matrices are built on DVE, the exp/broadcast work runs on the
    otherwise-idle Activation engine (wide w via stride-0-input Exp so the
    big weight multiply hits the DVE 2x bf16 mode; the narrow w is written
    by ACT directly into the matmul-rhs slot, eliminating a copy), and
    exact segment-sum runs as one-hot PE matmuls accumulating
    [sum w*h | sum w] per destination block. For heads==1 (layer 2) the
    edge weight is folded into the one-hot instead and the raw gathered
    rows (with a ones-column) serve as the matmul rhs directly.
  - Epilogue per block: divide by denominator, add bias, (layer 1: ELU via
    the Activation engine), write the core's output slice.

The host only does index preprocessing (graph partition / sort / padding),
weight repacking (folding attention vectors into the weight matrix:
W @ blockdiag(attn)), transposes of inputs, and concatenation of outputs.
All floating-point compute on the 800k edges / 50k nodes runs on device.
"""

import numpy as np
import ml_dtypes

BF16_NP = ml_dtypes.bfloat16

import concourse.bacc as bacc
import concourse.tile as tile
from concourse.tile_rust import add_dep_helper
from concourse import mybir
from concourse.bass_utils import run_bass_kernel_spmd

P = 128
N_CORES = 8
AluOp = mybir.AluOpType
ActFn = mybir.ActivationFunctionType
F32 = mybir.dt.float32
BF16 = mybir.dt.bfloat16
I16 = mybir.dt.int16

F_PREP = False  # SWDGE prepare_only + trigger_dma (desc-gen overlaps phase A)

# Problem constants (nn_GAT_Node_Classifier)
N_NODES = 50000
N_EDGES = 800000
IN_DIM = 256
HID = 32
HEADS = 8
OUT_DIM = 16
NEG_SLOPE = 0.2


class LayerCfg:
    def __init__(self, n_in, n_h, heads, elem, n_nodes, n_cores, split, elu,
                 out_bf16):
        self.n_in = n_in              # input feature dim (must be mult of 128)
        self.n_h = n_h                # heads * hid
        self.heads = heads
        self.hid = n_h // heads
        self.elem = elem              # gather row floats (>= n_h + heads, 256B mult)
        self.n_cores = n_cores
        self.nodes_per_core = n_nodes // n_cores
        self.blocks = (self.nodes_per_core + P - 1) // P
        self.npad = self.blocks * P
        self.n_nodes = n_nodes
        self.n_nodes_pad = ((n_nodes + P - 1) // P) * P
        self.nt = self.n_nodes_pad // P   # node tiles for the table build
        self.split = min(split, self.n_nodes_pad)
        self.elu = elu
        self.out_bf16 = out_bf16
        self.wcols = n_h + 2 * heads
        assert (self.elem * 2) % 256 == 0 and self.elem >= self.n_h + self.heads
        assert self.split % P == 0


class EdgePlan:
    """Host-side edge structures, uniform across cores (SPMD)."""

    def __init__(self, src, dst, cfg: LayerCfg):
        nc_, npc, blocks, split = cfg.n_cores, cfg.nodes_per_core, cfg.blocks, cfg.split
        src = np.asarray(src, dtype=np.int64)
        dst = np.asarray(dst, dtype=np.int64)
        core = dst // npc
        dstl = dst - core * npc
        blk = dstl // P
        dstb = (dstl - blk * P).astype(np.float32)
        grp = (src >= split).astype(np.int64)
        key = (core * blocks + blk) * 2 + grp
        order = np.argsort(key, kind="stable")
        cnt = np.bincount(key, minlength=nc_ * blocks * 2).reshape(nc_, blocks, 2)
        T = -(-cnt.max(axis=0) // P)          # [blocks, 2] tiles per block/group
        T[:, 0] = np.maximum(T[:, 0], 1)      # guarantee psum init per block
        self.T = T
        self.TL = int(T[:, 0].sum())
        self.TH = int(T[:, 1].sum())
        self.T_ALL = self.TL + self.TH
        self.lo_start = np.concatenate([[0], np.cumsum(T[:, 0])])[:-1]
        self.hi_start = np.concatenate([[0], np.cumsum(T[:, 1])])[:-1]
        ptr = np.concatenate([[0], np.cumsum(cnt.reshape(-1))])

        idx_lo = np.zeros((nc_, self.TL * P), np.int16)
        idx_hi = np.zeros((nc_, max(self.TH, 1) * P), np.int16)
        dstc = np.full((nc_, self.T_ALL * P), -1.0, np.float32)
        for c in range(nc_):
            for b in range(blocks):
                for g in range(2):
                    k = (c * blocks + b) * 2 + g
                    e0, e1 = int(ptr[k]), int(ptr[k + 1])
                    if e1 == e0:
                        continue
                    eidx = order[e0:e1]
                    n = e1 - e0
                    if g == 0:
                        base = int(self.lo_start[b]) * P
                        idx_lo[c, base:base + n] = (src[eidx]).astype(np.int16)
                        sbase = base
                    else:
                        base = int(self.hi_start[b]) * P
                        idx_hi[c, base:base + n] = (src[eidx] - split).astype(np.int16)
                        sbase = self.TL * P + base
                    dstc[c, sbase:sbase + n] = dstb[eidx]

        # gather instruction split (1024 idx each, tail partial)
        def gsizes(n_tiles):
            out = []
            rem = n_tiles
            while rem > 0:
                t = min(8, rem)
                out.append(t * P)
                rem -= t
            return out

        self.glo_sizes = gsizes(self.TL)
        self.ghi_sizes = gsizes(self.TH)

        def wrap_idx(flat_all, sizes):
            ng = max(len(sizes), 1)
            out = np.zeros((nc_, P, ng * 64), np.int16)
            for c in range(nc_):
                pos = 0
                for gi, ni in enumerate(sizes):
                    blk16 = flat_all[c, pos:pos + ni].reshape(ni // 16, 16).T
                    out[c, :, gi * 64: gi * 64 + ni // 16] = np.tile(blk16, (8, 1))
                    pos += ni
            return out

        self.idx_lo_w = wrap_idx(idx_lo, self.glo_sizes)
        self.idx_hi_w = wrap_idx(idx_hi, self.ghi_sizes)

        # dstc columns [128, T_ALL]: column t = dst-in-block of the tile's edges
        self.dstc_cols = dstc.reshape(nc_, self.T_ALL, P).transpose(0, 2, 1).copy()

        # dstT2: one row per gather (lo gathers then hi gathers): the gather's
        # edge destinations laid out along the free dim, for the K=1 PE
        # broadcast matmul.
        self.n_gathers = len(self.glo_sizes) + len(self.ghi_sizes)
        self.dstT2 = np.full((nc_, max(self.n_gathers, 1), 1024), -1.0, np.float32)
        gq = 0
        pos = 0
        for ni in self.glo_sizes + self.ghi_sizes:
            self.dstT2[:, gq, 0:ni] = dstc[:, pos:pos + ni]
            pos += ni
            gq += 1

        # per-block tile lists: (stream, stream_tile_idx, abs_t)
        self.block_tiles = []
        for b in range(blocks):
            tl = [("lo", int(self.lo_start[b]) + i, int(self.lo_start[b]) + i)
                  for i in range(int(T[b, 0]))]
            th = [("hi", int(self.hi_start[b]) + i, self.TL + int(self.hi_start[b]) + i)
                  for i in range(int(T[b, 1]))]
            self.block_tiles.append(tl + th)
        # map stream tile -> block
        self.tile_block = {}
        for b, tl in enumerate(self.block_tiles):
            for (s, t, a) in tl:
                self.tile_block[(s, t)] = b


def build_layer(cfg: LayerCfg, plan: EdgePlan):
    nc = bacc.Bacc("TRN2", target_bir_lowering=False, debug=False,
                   num_devices=cfg.n_cores, dynamic_dma_scratch_size=65536,
                   num_swdge_queues=4)
    n_in, n_h, heads, hid = cfg.n_in, cfg.n_h, cfg.heads, cfg.hid
    elem, wcols = cfg.elem, cfg.wcols
    kchunks = n_in // P
    NL = cfg.split
    NH = cfg.n_nodes_pad - cfg.split
    OUTDT = BF16 if cfg.out_bf16 else F32
    nf = n_h + heads  # [w*h | w] width (heads>1) / [h | w-slot] (heads==1)
    # table row: heads>1 -> [h | el]; heads==1 -> [h | 1.0 | el] (the ones
    # column makes the raw gathered rows usable as the accum matmul rhs)
    ecol = n_h + (1 if heads == 1 else 0)
    twid = ecol + heads

    xT = nc.dram_tensor("xT", [n_in, cfg.n_nodes_pad], BF16, kind="ExternalInput").ap()
    xT_own = nc.dram_tensor("xT_own", [n_in, cfg.npad], BF16, kind="ExternalInput").ap()
    Wcat = nc.dram_tensor("Wcat", [P, kchunks * wcols], BF16, kind="ExternalInput").ap()
    bias_rep = nc.dram_tensor("bias_rep", [P, n_h], F32, kind="ExternalInput").ap()
    iota_row = nc.dram_tensor("iota_row", [P, P], BF16, kind="ExternalInput").ap()
    iota_col = nc.dram_tensor("iota_col", [P, 1], F32, kind="ExternalInput").ap()
    ones_row = nc.dram_tensor("ones_row", [1, P], BF16, kind="ExternalInput").ap()
    nglo = max(len(plan.glo_sizes), 1)
    nghi = max(len(plan.ghi_sizes), 1)
    idx_lo = nc.dram_tensor("idx_lo", [P, nglo * 64], I16, kind="ExternalInput").ap()
    idx_hi = nc.dram_tensor("idx_hi", [P, nghi * 64], I16, kind="ExternalInput").ap()
    dstc = nc.dram_tensor("dstc", [P, plan.T_ALL], BF16, kind="ExternalInput").ap()
    dstT2 = nc.dram_tensor("dstT2", [max(plan.n_gathers, 1), 1024], BF16, kind="ExternalInput").ap()
    out_x = nc.dram_tensor("out_x", [cfg.npad, n_h], OUTDT, kind="ExternalOutput").ap()
    tab_lo = nc.dram_tensor("tab_lo", [NL, elem], BF16).ap()
    tab_hi = nc.dram_tensor("tab_hi", [max(NH, P), elem], BF16).ap()

    with tile.TileContext(nc) as tc:
        with tc.tile_pool(name="const", bufs=1) as cpool:
            Wcat_sb = cpool.tile([P, kchunks * wcols], BF16)
            nc.sync.dma_start(out=Wcat_sb[:], in_=Wcat[:])
            bias_sb = cpool.tile([P, n_h], F32)
            nc.sync.dma_start(out=bias_sb[:], in_=bias_rep[:])
            ir_sb = cpool.tile([P, P], BF16)
            nc.sync.dma_start(out=ir_sb[:], in_=iota_row[:])
            ic_sb = cpool.tile([P, 1], F32)
            nc.sync.dma_start(out=ic_sb[:], in_=iota_col[:])
            ones_sb = cpool.tile([1, P], BF16)
            nc.sync.dma_start(out=ones_sb[:], in_=ones_row[:])
            ixlo_sb = cpool.tile([P, nglo * 64], I16)
            nc.sync.dma_start(out=ixlo_sb[:], in_=idx_lo[:])
            ixhi_sb = cpool.tile([P, nghi * 64], I16)
            nc.sync.dma_start(out=ixhi_sb[:], in_=idx_hi[:])
            dstc_sb = cpool.tile([P, plan.T_ALL], BF16)
            nc.sync.dma_start(out=dstc_sb[:], in_=dstc[:])
            er_all = cpool.tile([P, cfg.blocks * heads], BF16)

            # ---- Phase A: projection tables ----
            GT = 4
            tab_writes = []
            with tc.tile_pool(name="pa_sb", bufs=3) as apool, \
                 tc.tile_pool(name="pa_ps", bufs=4, space="PSUM") as appool:
                assert NL % (GT * P) == 0
                for j0 in range(0, cfg.nt, GT):
                    gsz = min(GT, cfg.nt - j0)
                    xa = []
                    for k in range(kchunks):
                        t = apool.tile([P, GT * P], BF16, tag=f"x{k}")
                        nc.sync.dma_start(
                            out=t[:, 0:gsz * P],
                            in_=xT[k * P:(k + 1) * P, j0 * P:(j0 + gsz) * P])
                        xa.append(t)
                    stage = apool.tile([P, GT, twid], BF16, tag="stage")
                    if heads == 1:
                        nc.vector.memset(stage[:, :, n_h:n_h + 1], 1.0)
                    for jj in range(gsz):
                        ps = appool.tile([P, wcols], F32, tag="ps")
                        for k in range(kchunks):
                            nc.tensor.matmul(
                                out=ps[:], lhsT=xa[k][:, jj * P:(jj + 1) * P],
                                rhs=Wcat_sb[:, k * wcols:k * wcols + wcols],
                                start=(k == 0), stop=(k == kchunks - 1))
                        if jj % 2 == 0:
                            nc.scalar.copy(stage[:, jj, 0:n_h], ps[:, 0:n_h])
                            nc.scalar.copy(stage[:, jj, ecol:ecol + heads],
                                           ps[:, n_h:n_h + heads])
                        else:
                            nc.vector.tensor_copy(stage[:, jj, 0:n_h], ps[:, 0:n_h])
                            nc.vector.tensor_copy(stage[:, jj, ecol:ecol + heads],
                                                  ps[:, n_h:n_h + heads])
                    if j0 * P < NL:
                        dst_ap = tab_lo[j0 * P:(j0 + gsz) * P, 0:twid]
                    else:
                        r0 = j0 * P - NL
                        dst_ap = tab_hi[r0:r0 + gsz * P, 0:twid]
                    dst_ap = dst_ap.rearrange("(g p) c -> p g c", p=P)
                    tab_writes.append(
                        nc.sync.dma_start(out=dst_ap, in_=stage[:, 0:gsz, :]))
                # er for own nodes -> SBUF resident
                for b0 in range(0, cfg.blocks, GT):
                    gsz = min(GT, cfg.blocks - b0)
                    xa = []
                    for k in range(kchunks):
                        t = apool.tile([P, GT * P], BF16, tag=f"xo{k}")
                        nc.sync.dma_start(
                            out=t[:, 0:gsz * P],
                            in_=xT_own[k * P:(k + 1) * P, b0 * P:(b0 + gsz) * P])
                        xa.append(t)
                    for jj in range(gsz):
                        b = b0 + jj
                        ps = appool.tile([P, heads], F32, tag="pser")
                        for k in range(kchunks):
                            nc.tensor.matmul(
                                out=ps[:], lhsT=xa[k][:, jj * P:(jj + 1) * P],
                                rhs=Wcat_sb[:, k * wcols + n_h + heads:k * wcols + n_h + 2 * heads],
                                start=(k == 0), stop=(k == kchunks - 1))
                        nc.scalar.copy(er_all[:, b * heads:(b + 1) * heads], ps[:])

            # ---- Phase B: edge processing ----
            # fence: gather TRANSFERS must run after all phase-A table writes
            fence_tile = cpool.tile([1, 1], F32)
            fence = nc.vector.memset(fence_tile[:], 0.0)
            for wi in tab_writes:
                add_dep_helper(fence.ins, wi.ins, True, "gather tables written")
            qsems = [nc.alloc_semaphore(f"gsem{q}") for q in range(4)]
            with tc.tile_pool(name="glo", bufs=4) as glo_pool, \
                 tc.tile_pool(name="ghi", bufs=4) as ghi_pool, \
                 tc.tile_pool(name="sel", bufs=6) as sel_pool, \
                 tc.tile_pool(name="wg", bufs=4) as wg_pool, \
                 tc.tile_pool(name="sw", bufs=6) as sw_pool, \
                 tc.tile_pool(name="ep", bufs=3) as ep_pool, \
                 tc.tile_pool(name="ps_bc", bufs=4, space="PSUM") as bc_pool, \
                 tc.tile_pool(name="ps_er", bufs=2, space="PSUM") as er_pool, \
                 tc.tile_pool(name="ps_out", bufs=2, space="PSUM") as out_pool:

                group_data = {}

                def ensure_gather(strm, gi):
                    if (strm, gi) in group_data:
                        return group_data[(strm, gi)]
                    if strm == "lo":
                        ni = plan.glo_sizes[gi]
                        gq = gi
                        pool_, tab, ixsb = glo_pool, tab_lo, ixlo_sb
                    else:
                        ni = plan.ghi_sizes[gi]
                        gq = len(plan.glo_sizes) + gi
                        pool_, tab, ixsb = ghi_pool, tab_hi, ixhi_sb
                    ngt = ni // P
                    q = gq % 4
                    buf = pool_.tile([P, 8, elem], BF16, tag="g" + strm)
                    if F_PREP:
                        nc.gpsimd.dma_gather(
                            buf[:, 0:ngt, :], tab[:],
                            ixsb[:, gi * 64:gi * 64 + ni // 16],
                            ni, ni, elem, queue_num=q,
                            prepare_only=True, sem=qsems[q])
                        trig = nc.gpsimd.trigger_dma(count=None, queue_num=q)
                        add_dep_helper(trig.ins, fence.ins, True, "gather after fence")
                    else:
                        gins = nc.gpsimd.dma_gather(
                            buf[:, 0:ngt, :], tab[:],
                            ixsb[:, gi * 64:gi * 64 + ni // 16],
                            ni, ni, elem, queue_num=q)
                        add_dep_helper(gins.ins, fence.ins, True, "gather after fence")
                        trig = gins
                    # broadcast dst rows for this gather: psum_bc[n, e] = dst[e]
                    dr = sw_pool.tile([1, 1024], BF16, tag="dr")
                    nc.sync.dma_start(out=dr[:, 0:ni], in_=dstT2[gq:gq + 1, 0:ni])
                    # SEL^T via half-width (1-bank) bc tiles: doubles the PSUM
                    # pipeline depth of the stage that heads the gather chain
                    selt8 = sel_pool.tile([P, 1024], BF16, tag="selt")
                    for h in range(0, ni, 512):
                        w = min(512, ni - h)
                        bc = bc_pool.tile([P, 512], F32, tag="bc")
                        nc.tensor.matmul(
                            out=bc[:, 0:w], lhsT=ones_sb[:],
                            rhs=dr[:, h:h + w],
                            start=True, stop=True)
                        nc.vector.tensor_scalar(
                            selt8[:, h:h + w], bc[:, 0:w], ic_sb[:], None,
                            AluOp.is_equal)
                    # SEL: one-hot along free (node) axis per tile
                    t0_abs = plan.abs_base[(strm, gi)]
                    sel8 = sel_pool.tile([P, 8, P], BF16, tag="sel8")
                    nc.vector.tensor_tensor(
                        out=sel8[:, 0:ngt, :],
                        in0=ir_sb[:].unsqueeze(1).to_broadcast([P, ngt, P]),
                        in1=dstc_sb[:, t0_abs:t0_abs + ngt].unsqueeze(2).to_broadcast(
                            [P, ngt, P]),
                        op=AluOp.is_equal)
                    # er per edge: er8[:, r*heads:...] = selt8_r^T-matmul er_blk
                    er8 = er_pool.tile([P, 8 * heads], F32, tag="er8")
                    for r in range(ngt):
                        bb = plan.tile_block[(strm, gi * 8 + r)]
                        nc.tensor.matmul(
                            out=er8[:, r * heads:(r + 1) * heads],
                            lhsT=selt8[:, r * P:(r + 1) * P],
                            rhs=er_all[:, bb * heads:(bb + 1) * heads],
                            start=True, stop=True)
                    # s = el + er ; w = exp(leaky_relu(s))
                    s8 = sw_pool.tile([P, 8 * heads], F32, tag="s8")
                    s8_op = nc.vector.tensor_tensor(
                        out=s8[:, 0:ngt * heads].rearrange("p (t h) -> p t h", t=ngt),
                        in0=er8[:, 0:ngt * heads].rearrange("p (t h) -> p t h", t=ngt),
                        in1=buf[:, 0:ngt, ecol:ecol + heads], op=AluOp.add)
                    lr8 = sw_pool.tile([P, 8 * heads], F32, tag="lr8")
                    nc.vector.scalar_tensor_tensor(
                        lr8[:, 0:ngt * heads], s8[:, 0:ngt * heads], NEG_SLOPE,
                        s8[:, 0:ngt * heads], AluOp.mult, AluOp.max)
                    # wgws: [w*h | w] per tile; w written by ACT into the slot
                    wgws = wg_pool.tile([P, 8, nf], BF16, tag="wgws")
                    nc.scalar.activation(
                        wgws[:, 0:ngt, n_h:nf],
                        lr8[:, 0:ngt * heads].rearrange("p (t h) -> p t h", t=ngt),
                        ActFn.Exp)
                    if heads > 1:
                        # wide w on ACT (stride-0 broadcast input), then a
                        # 2x-mode bf16 multiply on DVE
                        w8w = wg_pool.tile([P, 8, n_h], BF16, tag="w8w")
                        nc.scalar.activation(
                            w8w[:, 0:ngt, :].rearrange(
                                "p t (h d) -> p t h d", h=heads),
                            lr8[:, 0:ngt * heads].rearrange(
                                "p (t h) -> p t h", t=ngt).unsqueeze(3).to_broadcast(
                                [P, ngt, heads, hid]),
                            ActFn.Exp)
                        wg_op = nc.vector.tensor_tensor(
                            out=wgws[:, 0:ngt, 0:n_h],
                            in0=buf[:, 0:ngt, 0:n_h],
                            in1=w8w[:, 0:ngt, :], op=AluOp.mult)
                        group_data[(strm, gi)] = (sel8, wgws, trig, (wg_op, s8_op))
                    else:
                        # heads==1: fold w into the one-hot instead (rhs is the
                        # raw gathered rows, which carry a ones-column)
                        sel8w = sel_pool.tile([P, 8, P], BF16, tag="sel8w")
                        wg_op = nc.vector.tensor_tensor(
                            out=sel8w[:, 0:ngt, :],
                            in0=sel8[:, 0:ngt, :],
                            in1=wgws[:, 0:ngt, n_h:nf].to_broadcast([P, ngt, P]),
                            op=AluOp.mult)
                        group_data[(strm, gi)] = (sel8w, buf, trig, (wg_op, s8_op))
                    return group_data[(strm, gi)]

                nrhs = nf if heads > 1 else n_h + 1
                for b in range(cfg.blocks):
                    tiles = plan.block_tiles[b]
                    pout = out_pool.tile([P, nrhs], F32, tag="pout")
                    for i, (strm, t, abs_t) in enumerate(tiles):
                        gi, r = divmod(t, 8)
                        sel, rhs8, _, _ = ensure_gather(strm, gi)
                        st = (i == 0)
                        sp = (i == len(tiles) - 1)
                        if heads > 1:
                            rhs = rhs8[:, r, :]
                        else:
                            rhs = rhs8[:, r, 0:n_h + 1]
                        nc.tensor.matmul(
                            out=pout[:], lhsT=sel[:, r, :],
                            rhs=rhs, start=st, stop=sp)
                    # epilogue
                    if heads > 1:
                        dnv = pout[:, n_h:n_h + heads]
                    else:
                        dnv = pout[:, n_h:n_h + 1]
                    dn = ep_pool.tile([P, heads], F32, tag="dn")
                    nc.vector.tensor_scalar_add(dn[:], dnv, 1e-30)
                    rec = ep_pool.tile([P, heads], F32, tag="rec")
                    nc.vector.reciprocal(rec[:], dn[:])
                    ox = ep_pool.tile([P, n_h], F32, tag="ox")
                    nc.vector.tensor_tensor(
                        out=ox[:].rearrange("p (h d) -> p h d", h=heads),
                        in0=pout[:, 0:n_h].rearrange("p (h d) -> p h d", h=heads),
                        in1=rec[:].unsqueeze(2).to_broadcast([P, heads, hid]),
                        op=AluOp.mult)
                    nc.vector.tensor_add(ox[:], ox[:], bias_sb[:])
                    if cfg.elu:
                        # elu(x) = relu(x) + exp(-relu(-x)) - 1, via ACT engine
                        oo = ep_pool.tile([P, n_h], OUTDT, tag="oo")
                        rp = ep_pool.tile([P, n_h], F32, tag="rp")
                        nc.scalar.activation(rp[:], ox[:], ActFn.Relu)
                        rn = ep_pool.tile([P, n_h], F32, tag="rn")
                        nc.scalar.activation(rn[:], ox[:], ActFn.Relu, scale=-1.0)
                        em = ep_pool.tile([P, n_h], F32, tag="em")
                        nc.scalar.activation(em[:], rn[:], ActFn.Exp, scale=-1.0)
                        nc.vector.scalar_tensor_tensor(
                            oo[:], em[:], -1.0, rp[:], AluOp.add, AluOp.add)
                    else:
                        oo = ox
                    nc.sync.dma_start(out=out_x[b * P:(b + 1) * P, :], in_=oo[:])

    nc.compile()
    return nc


def _prep_layer_inputs(cfg: LayerCfg, plan: EdgePlan, x_full, W, attn_l, attn_r, bias):
    """x_full: [n_nodes, n_in] float. Returns list of per-core input dicts."""
    n_in, n_h, heads, hid = cfg.n_in, cfg.n_h, cfg.heads, cfg.hid
    wcols = cfg.wcols
    # fold attention vectors: Wl = W @ blockdiag(attn)
    Bl = np.zeros((n_h, heads), np.float32)
    Br = np.zeros((n_h, heads), np.float32)
    for h in range(heads):
        Bl[h * hid:(h + 1) * hid, h] = attn_l[h]
        Br[h * hid:(h + 1) * hid, h] = attn_r[h]
    Wc = np.concatenate([W, W @ Bl, W @ Br], axis=1).astype(np.float32)  # [n_in, wcols]
    kchunks = n_in // P
    Wcat_host = Wc.reshape(kchunks, P, wcols).transpose(1, 0, 2).reshape(P, kchunks * wcols)
    Wcat_host = np.ascontiguousarray(Wcat_host).astype(BF16_NP)

    x_full = np.asarray(x_full, np.float32)
    xT = np.zeros((n_in, cfg.n_nodes_pad), BF16_NP)
    xT[:, 0:cfg.n_nodes] = np.ascontiguousarray(x_full.T)

    bias_r = np.ascontiguousarray(np.tile(bias.reshape(1, n_h), (P, 1)).astype(np.float32))
    iota_row = np.ascontiguousarray(
        np.tile(np.arange(P, dtype=np.float32).reshape(1, P), (P, 1))).astype(BF16_NP)
    iota_col = np.ascontiguousarray(np.arange(P, dtype=np.float32).reshape(P, 1))
    ones_row = np.ones((1, P), BF16_NP)

    ins = []
    npc = cfg.nodes_per_core
    for c in range(cfg.n_cores):
        xo = np.zeros((n_in, cfg.npad), BF16_NP)
        lo = c * npc
        hi = min((c + 1) * npc, cfg.n_nodes)
        xo[:, 0:hi - lo] = x_full[lo:hi].T
        ins.append({
            "xT": xT,
            "xT_own": np.ascontiguousarray(xo),
            "Wcat": Wcat_host,
            "bias_rep": bias_r,
            "iota_row": iota_row,
            "iota_col": iota_col,
            "ones_row": ones_row,
            "idx_lo": np.ascontiguousarray(plan.idx_lo_w[c]),
            "idx_hi": np.ascontiguousarray(plan.idx_hi_w[c]),
            "dstc": np.ascontiguousarray(plan.dstc_cols[c]).astype(BF16_NP),
            "dstT2": np.ascontiguousarray(plan.dstT2[c]).astype(BF16_NP),
        })
    return ins


def run_gat(emb, src, dst, W1, attn_l1, attn_r1, bias1, W2, attn_l2, attn_r2, bias2,
            n_nodes=N_NODES, split=32768, trace=False, tmpdir=None):
    emb = np.asarray(emb, np.float32)
    n_in = emb.shape[1]
    cfg1 = LayerCfg(n_in, HEADS * HID, HEADS, 384, n_nodes, N_CORES, split,
                    elu=True, out_bf16=False)
    cfg2 = LayerCfg(HEADS * HID, OUT_DIM, 1, 128, n_nodes, N_CORES, split,
                    elu=False, out_bf16=False)
    plan = EdgePlan(src, dst, cfg1)
    # annotate abs base col per gather group (for SEL8 build)
    plan.abs_base = {}
    pos = 0
    for gi, ni in enumerate(plan.glo_sizes):
        plan.abs_base[("lo", gi)] = pos
        pos += ni // P
    for gi, ni in enumerate(plan.ghi_sizes):
        plan.abs_base[("hi", gi)] = pos
        pos += ni // P

    nc1 = build_layer(cfg1, plan)
    ins1 = _prep_layer_inputs(cfg1, plan, emb, np.asarray(W1, np.float32),
                              np.asarray(attn_l1, np.float32),
                              np.asarray(attn_r1, np.float32),
                              np.asarray(bias1, np.float32))
    res1 = run_bass_kernel_spmd(nc1, ins1, list(range(N_CORES)), trace=trace,
                                tmpdir=None if tmpdir is None else tmpdir + "_l1")
    npc = cfg1.nodes_per_core
    x2 = np.concatenate(
        [res1.results[c]["out_x"][0:min(npc, n_nodes - c * npc)].astype(np.float32)
         for c in range(N_CORES)],
        axis=0)  # [n_nodes, 256]

    # layer-2 gather rows: [h2 (16) | 1.0 | el2 ...]; reorder Wcat cols so the
    # layer-2 projection emits [h2 | el | er] and phase A stages [h2 | 1 | el].
    nc2 = build_layer(cfg2, plan)
    ins2 = _prep_layer_inputs(cfg2, plan, x2, np.asarray(W2, np.float32),
                              np.asarray(attn_l2, np.float32).reshape(1, OUT_DIM),
                              np.asarray(attn_r2, np.float32).reshape(1, OUT_DIM),
                              np.asarray(bias2, np.float32))
    res2 = run_bass_kernel_spmd(nc2, ins2, list(range(N_CORES)), trace=trace,
                                tmpdir=None if tmpdir is None else tmpdir + "_l2")
    out = np.concatenate(
        [res2.results[c]["out_x"][0:min(npc, n_nodes - c * npc)] for c in range(N_CORES)],
        axis=0)
    exec_ns = [res1.exec_time_ns, res2.exec_time_ns]
    return out.astype(np.float32), exec_ns


def kernel(emb, src, dst, W1, attn_l1, attn_r1, bias1, W2, attn_l2, attn_r2, bias2):
    out, _ = run_gat(emb, src, dst, W1, attn_l1, attn_r1, bias1,
                     W2, attn_l2, attn_r2, bias2)
    return out
